# revision 35
# baseline (speedup 1.0000x reference)
"""Trainium2 Bass kernel for nn_Attention_Module (bilinear point sampling + attention).

Computation (see reference):
  a   = sigmoid(pts @ fc_w.T) * att_param; a = a / a   (dim-1 normalize of [N,1])
  out = bilinear_sample(img[B,C,H,W], pts) * a         -> [B, C, N]

Sharding: points are bucketed by floor(y)>>6 into 8 horizontal 64-row image
bands, one band per NeuronCore.  Each core receives its band in a channels-last
row-pair-interleaved layout [64, 512, 2, 256] (rows r and r+1 side by side) so
all four bilinear corners of a point form one contiguous 1024-f32 window, and
gathers one window per point straight from HBM via gpsimd.dma_gather.  In-band
pixel index (y-64k)*512+x fits the gather's int16 index requirement.  The
bilinear blend runs on ScalarE (first product) + VectorE (three fused
scalar_tensor_tensor MACs) with per-partition scalar weights; `a` is an exact
1.0/NaN mask computed from the sigmoid-underflow threshold.  The host does
bucketing, layout packing, and the inverse permutation/final transpose.
"""

import os

import numpy as np

import concourse.bacc as bacc
import concourse.mybir as mybir
from concourse.bass_utils import run_bass_kernel_spmd
from concourse.library_config import mlp
from concourse.tile import TileContext
from concourse.ap import AP

F32 = mybir.dt.float32
I16 = mybir.dt.int16

LAST_EXEC_NS = None

H = W = 512
B, C = 4, 64
BC = B * C                      # 256 channels, flattened (b*64+c)
N_CORES = 8
BAND = 64                       # y0 rows owned by one core
SLICE_ROWS = BAND + 1           # +1 row so y1=y0+1 stays in-slice
SLOT = 128                      # points per "slot" (one partition sweep)
BLK = 8                         # slots per gather block (1024 points)

# z threshold below which the reference's sigmoid underflows to exactly 0,
# making a = 0/0 = NaN.  float32 exp(z) rounds to +0 for z < ln(2^-150).
SIGMOID_ZERO_CUT = -103.97207708399179


def _build_nc(w0: float, w1: float, n_slots: int, blocks: list[int]):
    """Build the SPMD Bass program. blocks[i] = number of slots in block i."""
    nc = bacc.Bacc("TRN2", target_bir_lowering=False)
    j128 = n_slots                       # columns in 128-wrap layouts
    j16 = n_slots * 8                    # columns in 16-wrap layouts

    # row-pair-interleaved image slice: [64 rows, 512 cols, 2 rows, 256 ch].
    # All four bilinear corners of a point are one contiguous 1024-f32 window.
    img = nc.declare_dram_parameter("img", [BAND * W * 2 * BC], F32, isOutput=False)
    # x-plane then y-plane, 128-wrap (point j -> [j%128, j//128])
    p128 = nc.declare_dram_parameter("p128", [128, 2 * j128], F32, isOutput=False)
    # y_rel-plane then x-plane, 16-wrap replicated x8 (point j -> [j%16, j//16])
    p16 = nc.declare_dram_parameter("p16", [128, 2 * j16], F32, isOutput=False)
    attp = nc.declare_dram_parameter("attp", [128, j128], F32, isOutput=False)
    out_img = nc.declare_dram_parameter("out_img", [128, j128 * BC], F32, isOutput=True)
    out_a = nc.declare_dram_parameter("out_a", [128, j128], F32, isOutput=True)

    # gather table: rows are pixel-pair columns (stride 512 f32), each window
    # is 4 corner pixels = 1024 f32.  32767 rows: max used index is
    # 63*512+509 = 32765; 512*32766+1024 exactly equals the slice size.
    table = AP(img, 0, [(2 * BC, 32767), (1, 4 * BC)])

    mult = mybir.AluOpType.mult
    add = mybir.AluOpType.add
    sub = mybir.AluOpType.subtract
    is_gt = mybir.AluOpType.is_gt
    is_ge = mybir.AluOpType.is_ge
    R2I = 8388608.0   # 2^23: (x + 2^23) - 2^23 rounds x to nearest int

    with TileContext(nc) as tc:
        nc.gpsimd.load_library(mlp)
        with (
            tc.tile_pool(name="const", bufs=1) as cpool,
            tc.tile_pool(name="gath", bufs=4) as gpool,
            tc.tile_pool(name="outp", bufs=3) as opool,
        ):
            t_p128 = cpool.tile([128, 2 * j128], F32)
            nc.sync.dma_start(t_p128[:, :], p128[:, :])
            t_p16 = cpool.tile([128, 2 * j16], F32)
            nc.sync.dma_start(t_p16[:, :], p16[:, :])
            t_att = cpool.tile([128, j128], F32)
            nc.sync.dma_start(t_att[:, :], attp[:, :])

            xs = t_p128[:, 0:j128]
            ys = t_p128[:, j128:2 * j128]

            # packed per-point scratch: fx, fy, gx, gy, wtl, wtr, wbl, wbr, z/a
            wk = cpool.tile([128, 10, j128], F32)
            fx = wk[:, 0, :]
            fy = wk[:, 1, :]
            gx = wk[:, 2, :]
            gy = wk[:, 3, :]
            wtl = wk[:, 4, :]
            wtr = wk[:, 5, :]
            wbl = wk[:, 6, :]
            wbr = wk[:, 7, :]
            zt = wk[:, 8, :]
            tm = wk[:, 9, :]

            def frac(out, v):
                # out = v - floor(v) for v in [0, 2^22), exact
                nc.vector.tensor_scalar(out, v, R2I, R2I, add, sub)  # round(v)
                nc.vector.tensor_tensor(tm, out, v, is_gt)           # over-round?
                nc.vector.tensor_tensor(out, out, tm, sub)           # floor(v)
                nc.vector.tensor_tensor(out, v, out, sub)            # frac

            # fractional parts and bilinear weights
            frac(fx, xs)
            frac(fy, ys)
            nc.vector.tensor_scalar(gx, fx, -1.0, 1.0, mult, add)   # 1-fx
            nc.vector.tensor_scalar(gy, fy, -1.0, 1.0, mult, add)   # 1-fy
            nc.vector.tensor_tensor(wtl, gx, gy, mult)
            nc.vector.tensor_tensor(wtr, fx, gy, mult)
            nc.vector.tensor_tensor(wbl, gx, fy, mult)
            nc.vector.tensor_tensor(wbr, fx, fy, mult)

            # attention a = sigmoid(x*w0 + y*w1) * att; a = a / a.
            # sigmoid > 0 always, so a == 1.0 exactly unless sigmoid*att == 0
            # (sigmoid underflow or att == 0), where a = 0/0 = NaN.  Build
            # that directly: m = (z >= cut), an = m*att, a = an/an.
            nc.vector.tensor_scalar(zt, xs, float(w0), None, mult)
            nc.vector.scalar_tensor_tensor(zt, ys, float(w1), zt, mult, add)
            sg = cpool.tile([128, 2, j128], F32)
            nc.vector.tensor_scalar(
                sg[:, 0, :], zt, float(SIGMOID_ZERO_CUT), None, is_ge)
            nc.vector.tensor_tensor(sg[:, 1, :], sg[:, 0, :], t_att[:, :], mult)
            av = sg[:, 0, :]
            # an/an via recip*mult: an is 0.0 or 1.0 here, so this is exactly
            # 1.0 for valid points and inf*0 = NaN for masked ones.
            nc.vector.reciprocal(av, sg[:, 1, :])
            nc.vector.tensor_tensor(av, av, sg[:, 1, :], mult)
            nc.sync.dma_start(out_a[:, :], av)
            # fold a into the bilinear weights: scales valid points by exactly
            # 1.0 (bit identity) and poisons masked points' output with NaN.
            nc.vector.tensor_tensor(wtl, wtl, av, mult)
            nc.vector.tensor_tensor(wtr, wtr, av, mult)
            nc.vector.tensor_tensor(wbl, wbl, av, mult)
            nc.vector.tensor_tensor(wbr, wbr, av, mult)

            # int16 gather indices: (floor(y_rel))*512 + floor(x), computed in
            # two passes: block 0's slice first so its gather launches early.
            scr = cpool.tile([128, 3, j16], F32)
            idx16 = cpool.tile([128, j16], I16)

            def idx_pass(c0, c1):
                yr = t_p16[:, c0:c1]
                xr = t_p16[:, j16 + c0: j16 + c1]
                t1 = scr[:, 0, c0:c1]
                t2 = scr[:, 1, c0:c1]
                acc = scr[:, 2, c0:c1]

                def floor_to(out, v):
                    nc.vector.tensor_scalar(out, v, R2I, R2I, add, sub)
                    nc.vector.tensor_tensor(t1, out, v, is_gt)
                    nc.vector.tensor_tensor(out, out, t1, sub)       # floor(v)

                floor_to(acc, yr)
                floor_to(t2, xr)
                nc.vector.scalar_tensor_tensor(acc, acc, float(W), t2, mult, add)
                nc.vector.tensor_copy(idx16[:, c0:c1], acc)

            first_cols = blocks[0] * 8
            idx_pass(0, first_cols)
            if first_cols < j16:
                idx_pass(first_cols, j16)

            # gather + blend, block by block
            stage = os.environ.get("KB_STAGE", "full")
            s_base = 0
            for ns in (blocks if stage != "pre" else []):
                npts = ns * SLOT
                g_t = gpool.tile([128, BLK, 4 * BC], F32, tag="ga")
                nc.gpsimd.dma_gather(
                    g_t[:, 0:ns, :], table,
                    idx16[:, s_base * 8: s_base * 8 + ns * 8], npts, npts,
                    4 * BC, elem_step=2 * BC, single_packet=False,
                )
                ot = opool.tile([128, BLK, BC], F32, tag="ot")
                if stage == "gather":
                    nc.vector.tensor_copy(ot[:, 0:ns, :], g_t[:, 0:ns, 0:BC])
                else:
                    # window layout: [TL 0:256, BL 256:512, TR 512:768, BR 768:1024]
                    # First product on ScalarE: keeps DVE in 1x mode (its fp32
                    # 2x tensor_scalar mode takes an SBUF-port lock that stalls
                    # the GpSimd gather descriptor generation).
                    for s in range(ns):
                        sc = s_base + s
                        o = ot[:, s, :]
                        nc.scalar.mul(o, g_t[:, s, 0:BC], wtl[:, sc:sc + 1])
                        nc.vector.scalar_tensor_tensor(
                            o, g_t[:, s, 2 * BC:3 * BC], wtr[:, sc:sc + 1], o, mult, add)
                        nc.vector.scalar_tensor_tensor(
                            o, g_t[:, s, BC:2 * BC], wbl[:, sc:sc + 1], o, mult, add)
                        nc.vector.scalar_tensor_tensor(
                            o, g_t[:, s, 3 * BC:4 * BC], wbr[:, sc:sc + 1], o, mult, add)
                nc.sync.dma_start(
                    out_img[:, s_base * BC:(s_base + ns) * BC], ot[:, 0:ns, :])
                s_base += ns
    nc.compile()
    return nc


def kernel(lidar_points, original_img, fc_w, att_param):
    pts = np.ascontiguousarray(np.asarray(lidar_points, dtype=np.float32))
    img = np.asarray(original_img, dtype=np.float32)
    fw = np.asarray(fc_w, dtype=np.float32)
    att = np.ascontiguousarray(np.asarray(att_param, dtype=np.float32))
    n = pts.shape[0]

    x = pts[:, 0]
    y = pts[:, 1]
    band = np.clip(np.floor(y).astype(np.int64) >> 6, 0, N_CORES - 1)
    order = np.argsort(band, kind="stable")
    counts = np.bincount(band, minlength=N_CORES)
    starts = np.zeros(N_CORES + 1, np.int64)
    np.cumsum(counts, out=starts[1:])

    # padded per-core point count (shared shape across cores)
    max_cnt = int(counts.max())
    n_slots = max(BLK, (max_cnt + SLOT - 1) // SLOT)
    p_pad = n_slots * SLOT
    # small first block so the gather pipeline starts flowing early (a longer
    # taper at either end was measured slower: tiny gathers pay the ~4us POOL
    # descriptor-generation fixed cost without filling DMA bandwidth)
    blocks = [2]
    rest = n_slots - 2
    blocks += [BLK] * (rest // BLK)
    if rest % BLK:
        blocks.append(rest % BLK)

    # channels-last image [H, W, BC]; channel index = b*C + c
    img_cl = np.ascontiguousarray(img.transpose(2, 3, 0, 1)).reshape(H, W, BC)

    in_maps = []
    for k in range(N_CORES):
        sel = order[starts[k]:starts[k + 1]]
        cnt = sel.size
        p_pts = np.zeros((p_pad, 2), np.float32)
        p_pts[:cnt] = pts[sel]
        p_pts[cnt:, 1] = 64.0 * k          # pad y inside the band -> idx 0
        p_att = np.ones((p_pad,), np.float32)
        p_att[:cnt] = att[sel, 0]

        j128 = n_slots
        w128 = p_pts.reshape(j128, 128, 2).transpose(2, 0, 1)   # [2, j128, 128]
        p128 = np.empty((128, 2 * j128), np.float32)
        p128[:, :j128] = w128[0].T                              # x plane
        p128[:, j128:] = w128[1].T                              # y plane

        y_rel = p_pts[:, 1] - np.float32(64.0 * k)
        j16 = n_slots * 8
        w16 = np.empty((16, 2 * j16), np.float32)
        w16[:, :j16] = y_rel.reshape(j16, 16).T
        w16[:, j16:] = p_pts[:, 0].reshape(j16, 16).T
        p16 = np.tile(w16, (8, 1))

        # row-pair interleave: img2[r, x, 0] = row r0+r, img2[r, x, 1] = row r0+r+1
        img2 = np.zeros((BAND, W, 2, BC), np.float32)
        r0 = BAND * k
        r1 = min(H, r0 + SLICE_ROWS)
        img2[:, :, 0, :] = img_cl[r0:r0 + BAND]
        img2[: r1 - r0 - 1, :, 1, :] = img_cl[r0 + 1:r1]

        in_maps.append({
            "img": img2.reshape(-1),
            "p128": p128,
            "p16": p16,
            "attp": np.ascontiguousarray(p_att.reshape(j128, 128).T),
        })

    nc = _build_nc(float(fw[0, 0]), float(fw[0, 1]), n_slots, blocks)
    trace = bool(int(os.environ.get("KERNEL_TRACE", "0")))
    res = run_bass_kernel_spmd(nc, in_maps, list(range(N_CORES)), trace=trace)
    global LAST_EXEC_NS
    LAST_EXEC_NS = res.exec_time_ns

    a_full = np.empty((n, 1), np.float32)
    out_pm = np.empty((n, BC), np.float32)
    for k in range(N_CORES):
        sel = order[starts[k]:starts[k + 1]]
        cnt = sel.size
        r = res.results[k]
        a_dev = r["out_a"]                    # [128, j128]
        a_full[sel, 0] = a_dev.T.reshape(-1)[:cnt]
        o_dev = r["out_img"].reshape(128, n_slots, BC)
        out_pm[sel] = o_dev.transpose(1, 0, 2).reshape(-1, BC)[:cnt]

    attended = np.ascontiguousarray(out_pm.T).reshape(B, C, n)
    return a_full, attended


# revision 36
# speedup vs baseline: 1.1886x; 1.1886x over previous
"""Trainium2 Bass kernel for nn_Attention_Module (bilinear point sampling + attention).

Computation (see reference):
  a   = sigmoid(pts @ fc_w.T) * att_param; a = a / a   (dim-1 normalize of [N,1])
  out = bilinear_sample(img[B,C,H,W], pts) * a         -> [B, C, N]

Sharding: points are bucketed by floor(y)>>6 into 8 horizontal 64-row image
bands, one band per NeuronCore.  Each core receives its band in a channels-last
row-pair-interleaved layout [64, 512, 2, 256] (rows r and r+1 side by side) so
all four bilinear corners of a point form one contiguous 1024-f32 window, and
gathers one window per point straight from HBM via gpsimd.dma_gather.  In-band
pixel index (y-64k)*512+x fits the gather's int16 index requirement.  The
bilinear blend runs on ScalarE (first product) + VectorE (three fused
scalar_tensor_tensor MACs) with per-partition scalar weights; `a` is an exact
1.0/NaN mask computed from the sigmoid-underflow threshold.  The host does
bucketing, layout packing, and the inverse permutation/final transpose.
"""

import os

import numpy as np

import concourse.bacc as bacc
import concourse.mybir as mybir
from concourse.bass_utils import run_bass_kernel_spmd
from concourse.library_config import mlp
from concourse.tile import TileContext
from concourse.ap import AP

F32 = mybir.dt.float32
I16 = mybir.dt.int16

LAST_EXEC_NS = None

H = W = 512
B, C = 4, 64
BC = B * C                      # 256 channels, flattened (b*64+c)
N_CORES = 8
BAND = 64                       # y0 rows owned by one core
SLICE_ROWS = BAND + 1           # +1 row so y1=y0+1 stays in-slice
SLOT = 128                      # points per "slot" (one partition sweep)
BLK = 8                         # slots per gather block (1024 points)

# z threshold below which the reference's sigmoid underflows to exactly 0,
# making a = 0/0 = NaN.  float32 exp(z) rounds to +0 for z < ln(2^-150).
SIGMOID_ZERO_CUT = -103.97207708399179


def _build_nc(w0: float, w1: float, n_slots: int, blocks: list[int]):
    """Build the SPMD Bass program. blocks[i] = number of slots in block i."""
    nc = bacc.Bacc("TRN2", target_bir_lowering=False)
    j128 = n_slots                       # columns in 128-wrap layouts
    j16 = n_slots * 8                    # columns in 16-wrap layouts

    # row-pair-interleaved image slice: [64 rows, 512 cols, 2 rows, 256 ch].
    # All four bilinear corners of a point are one contiguous 1024-f32 window.
    img = nc.declare_dram_parameter("img", [BAND * W * 2 * BC], F32, isOutput=False)
    # x-plane then y-plane, 128-wrap (point j -> [j%128, j//128])
    p128 = nc.declare_dram_parameter("p128", [128, 2 * j128], F32, isOutput=False)
    # y_rel-plane then x-plane, 16-wrap replicated x8 (point j -> [j%16, j//16])
    p16 = nc.declare_dram_parameter("p16", [128, 2 * j16], F32, isOutput=False)
    attp = nc.declare_dram_parameter("attp", [128, j128], F32, isOutput=False)
    out_img = nc.declare_dram_parameter("out_img", [128, j128 * BC], F32, isOutput=True)
    out_a = nc.declare_dram_parameter("out_a", [128, j128], F32, isOutput=True)

    # gather table: rows are pixel-pair columns (stride 512 f32), each window
    # is 4 corner pixels = 1024 f32.  32767 rows: max used index is
    # 63*512+509 = 32765; 512*32766+1024 exactly equals the slice size.
    table = AP(img, 0, [(2 * BC, 32767), (1, 4 * BC)])

    mult = mybir.AluOpType.mult
    add = mybir.AluOpType.add
    sub = mybir.AluOpType.subtract
    is_gt = mybir.AluOpType.is_gt
    is_ge = mybir.AluOpType.is_ge
    R2I = 8388608.0   # 2^23: (x + 2^23) - 2^23 rounds x to nearest int

    with TileContext(nc) as tc:
        nc.gpsimd.load_library(mlp)
        with (
            tc.tile_pool(name="const", bufs=1) as cpool,
            tc.tile_pool(name="gath", bufs=4) as gpool,
            tc.tile_pool(name="outp", bufs=3) as opool,
        ):
            # p16 lands in two pieces: block 0's index columns first, so the
            # first gather's descriptor generation starts ~3us earlier.
            c0 = blocks[0] * 8
            t_p16 = cpool.tile([128, 2 * j16], F32)
            nc.sync.dma_start(t_p16[:, 0:c0], p16[:, 0:c0])
            nc.sync.dma_start(
                t_p16[:, j16:j16 + c0], p16[:, j16:j16 + c0])
            nc.sync.dma_start(t_p16[:, c0:j16], p16[:, c0:j16])
            nc.sync.dma_start(
                t_p16[:, j16 + c0:2 * j16], p16[:, j16 + c0:2 * j16])
            t_p128 = cpool.tile([128, 2 * j128], F32)
            nc.sync.dma_start(t_p128[:, :], p128[:, :])
            t_att = cpool.tile([128, j128], F32)
            nc.sync.dma_start(t_att[:, :], attp[:, :])

            xs = t_p128[:, 0:j128]
            ys = t_p128[:, j128:2 * j128]

            # packed per-point scratch: fx, fy, gx, gy, wtl, wtr, wbl, wbr, z/a
            wk = cpool.tile([128, 10, j128], F32)
            fx = wk[:, 0, :]
            fy = wk[:, 1, :]
            gx = wk[:, 2, :]
            gy = wk[:, 3, :]
            wtl = wk[:, 4, :]
            wtr = wk[:, 5, :]
            wbl = wk[:, 6, :]
            wbr = wk[:, 7, :]
            zt = wk[:, 8, :]
            tm = wk[:, 9, :]

            def frac(out, v):
                # out = v - floor(v) for v in [0, 2^22), exact
                nc.vector.tensor_scalar(out, v, R2I, R2I, add, sub)  # round(v)
                nc.vector.tensor_tensor(tm, out, v, is_gt)           # over-round?
                nc.vector.tensor_tensor(out, out, tm, sub)           # floor(v)
                nc.vector.tensor_tensor(out, v, out, sub)            # frac

            # fractional parts and bilinear weights
            frac(fx, xs)
            frac(fy, ys)
            nc.vector.tensor_scalar(gx, fx, -1.0, 1.0, mult, add)   # 1-fx
            nc.vector.tensor_scalar(gy, fy, -1.0, 1.0, mult, add)   # 1-fy
            nc.vector.tensor_tensor(wtl, gx, gy, mult)
            nc.vector.tensor_tensor(wtr, fx, gy, mult)
            nc.vector.tensor_tensor(wbl, gx, fy, mult)
            nc.vector.tensor_tensor(wbr, fx, fy, mult)

            # attention a = sigmoid(x*w0 + y*w1) * att; a = a / a.
            # sigmoid > 0 always, so a == 1.0 exactly unless sigmoid*att == 0
            # (sigmoid underflow or att == 0), where a = 0/0 = NaN.  Build
            # that directly: m = (z >= cut), an = m*att, a = an/an.
            nc.vector.tensor_scalar(zt, xs, float(w0), None, mult)
            nc.vector.scalar_tensor_tensor(zt, ys, float(w1), zt, mult, add)
            sg = cpool.tile([128, 2, j128], F32)
            nc.vector.tensor_scalar(
                sg[:, 0, :], zt, float(SIGMOID_ZERO_CUT), None, is_ge)
            nc.vector.tensor_tensor(sg[:, 1, :], sg[:, 0, :], t_att[:, :], mult)
            av = sg[:, 0, :]
            # an/an via recip*mult: an is 0.0 or 1.0 here, so this is exactly
            # 1.0 for valid points and inf*0 = NaN for masked ones.
            nc.vector.reciprocal(av, sg[:, 1, :])
            nc.vector.tensor_tensor(av, av, sg[:, 1, :], mult)
            nc.sync.dma_start(out_a[:, :], av)
            # fold a into the bilinear weights: scales valid points by exactly
            # 1.0 (bit identity) and poisons masked points' output with NaN.
            nc.vector.tensor_tensor(wtl, wtl, av, mult)
            nc.vector.tensor_tensor(wtr, wtr, av, mult)
            nc.vector.tensor_tensor(wbl, wbl, av, mult)
            nc.vector.tensor_tensor(wbr, wbr, av, mult)

            # int16 gather indices: (floor(y_rel))*512 + floor(x), computed in
            # two passes: block 0's slice first so its gather launches early.
            scr = cpool.tile([128, 3, j16], F32)
            idx16 = cpool.tile([128, j16], I16)

            def idx_pass(c0, c1):
                yr = t_p16[:, c0:c1]
                xr = t_p16[:, j16 + c0: j16 + c1]
                t1 = scr[:, 0, c0:c1]
                t2 = scr[:, 1, c0:c1]
                acc = scr[:, 2, c0:c1]

                def floor_to(out, v):
                    nc.vector.tensor_scalar(out, v, R2I, R2I, add, sub)
                    nc.vector.tensor_tensor(t1, out, v, is_gt)
                    nc.vector.tensor_tensor(out, out, t1, sub)       # floor(v)

                floor_to(acc, yr)
                floor_to(t2, xr)
                nc.vector.scalar_tensor_tensor(acc, acc, float(W), t2, mult, add)
                nc.vector.tensor_copy(idx16[:, c0:c1], acc)

            first_cols = blocks[0] * 8
            idx_pass(0, first_cols)
            if first_cols < j16:
                idx_pass(first_cols, j16)

            # gather + blend, block by block
            stage = os.environ.get("KB_STAGE", "full")
            s_base = 0
            for ns in (blocks if stage != "pre" else []):
                npts = ns * SLOT
                g_t = gpool.tile([128, BLK, 4 * BC], F32, tag="ga")
                nc.gpsimd.dma_gather(
                    g_t[:, 0:ns, :], table,
                    idx16[:, s_base * 8: s_base * 8 + ns * 8], npts, npts,
                    4 * BC, elem_step=2 * BC, single_packet=False,
                )
                ot = opool.tile([128, BLK, BC], F32, tag="ot")
                if stage == "gather":
                    nc.vector.tensor_copy(ot[:, 0:ns, :], g_t[:, 0:ns, 0:BC])
                else:
                    # window layout: [TL 0:256, BL 256:512, TR 512:768, BR 768:1024]
                    # First product on ScalarE: keeps DVE in 1x mode (its fp32
                    # 2x tensor_scalar mode takes an SBUF-port lock that stalls
                    # the GpSimd gather descriptor generation).
                    for s in range(ns):
                        sc = s_base + s
                        o = ot[:, s, :]
                        nc.scalar.mul(o, g_t[:, s, 0:BC], wtl[:, sc:sc + 1])
                        nc.vector.scalar_tensor_tensor(
                            o, g_t[:, s, 2 * BC:3 * BC], wtr[:, sc:sc + 1], o, mult, add)
                        nc.vector.scalar_tensor_tensor(
                            o, g_t[:, s, BC:2 * BC], wbl[:, sc:sc + 1], o, mult, add)
                        nc.vector.scalar_tensor_tensor(
                            o, g_t[:, s, 3 * BC:4 * BC], wbr[:, sc:sc + 1], o, mult, add)
                nc.sync.dma_start(
                    out_img[:, s_base * BC:(s_base + ns) * BC], ot[:, 0:ns, :])
                s_base += ns
    nc.compile()
    return nc


def kernel(lidar_points, original_img, fc_w, att_param):
    pts = np.ascontiguousarray(np.asarray(lidar_points, dtype=np.float32))
    img = np.asarray(original_img, dtype=np.float32)
    fw = np.asarray(fc_w, dtype=np.float32)
    att = np.ascontiguousarray(np.asarray(att_param, dtype=np.float32))
    n = pts.shape[0]

    x = pts[:, 0]
    y = pts[:, 1]
    band = np.clip(np.floor(y).astype(np.int64) >> 6, 0, N_CORES - 1)
    order = np.argsort(band, kind="stable")
    counts = np.bincount(band, minlength=N_CORES)
    starts = np.zeros(N_CORES + 1, np.int64)
    np.cumsum(counts, out=starts[1:])

    # padded per-core point count (shared shape across cores)
    max_cnt = int(counts.max())
    n_slots = max(BLK, (max_cnt + SLOT - 1) // SLOT)
    p_pad = n_slots * SLOT
    # small first block so the gather pipeline starts flowing early (a longer
    # taper at either end was measured slower: tiny gathers pay the ~4us POOL
    # descriptor-generation fixed cost without filling DMA bandwidth)
    blocks = [2]
    rest = n_slots - 2
    blocks += [BLK] * (rest // BLK)
    if rest % BLK:
        blocks.append(rest % BLK)

    # channels-last image [H, W, BC]; channel index = b*C + c
    img_cl = np.ascontiguousarray(img.transpose(2, 3, 0, 1)).reshape(H, W, BC)

    in_maps = []
    for k in range(N_CORES):
        sel = order[starts[k]:starts[k + 1]]
        cnt = sel.size
        p_pts = np.zeros((p_pad, 2), np.float32)
        p_pts[:cnt] = pts[sel]
        p_pts[cnt:, 1] = 64.0 * k          # pad y inside the band -> idx 0
        p_att = np.ones((p_pad,), np.float32)
        p_att[:cnt] = att[sel, 0]

        j128 = n_slots
        w128 = p_pts.reshape(j128, 128, 2).transpose(2, 0, 1)   # [2, j128, 128]
        p128 = np.empty((128, 2 * j128), np.float32)
        p128[:, :j128] = w128[0].T                              # x plane
        p128[:, j128:] = w128[1].T                              # y plane

        y_rel = p_pts[:, 1] - np.float32(64.0 * k)
        j16 = n_slots * 8
        w16 = np.empty((16, 2 * j16), np.float32)
        w16[:, :j16] = y_rel.reshape(j16, 16).T
        w16[:, j16:] = p_pts[:, 0].reshape(j16, 16).T
        p16 = np.tile(w16, (8, 1))

        # row-pair interleave: img2[r, x, 0] = row r0+r, img2[r, x, 1] = row r0+r+1
        img2 = np.zeros((BAND, W, 2, BC), np.float32)
        r0 = BAND * k
        r1 = min(H, r0 + SLICE_ROWS)
        img2[:, :, 0, :] = img_cl[r0:r0 + BAND]
        img2[: r1 - r0 - 1, :, 1, :] = img_cl[r0 + 1:r1]

        in_maps.append({
            "img": img2.reshape(-1),
            "p128": p128,
            "p16": p16,
            "attp": np.ascontiguousarray(p_att.reshape(j128, 128).T),
        })

    nc = _build_nc(float(fw[0, 0]), float(fw[0, 1]), n_slots, blocks)
    trace = bool(int(os.environ.get("KERNEL_TRACE", "0")))
    res = run_bass_kernel_spmd(nc, in_maps, list(range(N_CORES)), trace=trace)
    global LAST_EXEC_NS
    LAST_EXEC_NS = res.exec_time_ns

    a_full = np.empty((n, 1), np.float32)
    out_pm = np.empty((n, BC), np.float32)
    for k in range(N_CORES):
        sel = order[starts[k]:starts[k + 1]]
        cnt = sel.size
        r = res.results[k]
        a_dev = r["out_a"]                    # [128, j128]
        a_full[sel, 0] = a_dev.T.reshape(-1)[:cnt]
        o_dev = r["out_img"].reshape(128, n_slots, BC)
        out_pm[sel] = o_dev.transpose(1, 0, 2).reshape(-1, BC)[:cnt]

    attended = np.ascontiguousarray(out_pm.T).reshape(B, C, n)
    return a_full, attended
